# revision 14
# baseline (speedup 1.0000x reference)
"""Self-contained Trainium2 Bass kernel for the fused attention layer.

Full computation:
    qkv = data @ W_qkv + b_qkv ; split into q,k,v heads (H=16, HD=64)
    scores = softmax(q k^T / sqrt(64)) ; out = (scores @ v) @ W_out + b_out

Sharding over 8 NeuronCores: core c = (batch n = c//2, head-group g = c%2).
Each core computes attention for 8 of the 16 heads of one batch and its
partial output projection; host sums the two partials per batch and adds
b_out.

On-chip layout per core (fp32 data, fp32r matmuls):
  XT  [1024, 2048]  x^T for this batch (feature-major => no transposes)
  QKT [1024, 2048]  (q|k)^T for this head group  = Wqk^T @ x
  V   [2048, 8*65]  v in natural orientation, per-head 64 cols + ones col
                    (ones column yields the softmax denominator Z during
                    the attention*V matmul)
  S^T tiles [128 keys, q] => exp on ACT (no max subtraction: |scores|<~3)
  AV^T psum [65, q] accumulated over key chunks; row 64 = Z
  normalization: PE broadcast of Z + DVE reciprocal/mul
  OUT^T [1024, 2048] partial (no bias), host adds b_out and transposes.

`reps` emits the whole pipeline N times back to back (used only by the
benchmark harness to wall-clock the steady-state iteration time through
the high-latency axon tunnel).
"""

import sys

for _p in ("/opt/trn_rl_repo",):
    if _p not in sys.path:
        sys.path.insert(0, _p)

import numpy as np

import concourse.bass as bass
import concourse.mybir as mybir
import concourse.tile as tile
from concourse import bacc
from concourse.bass_utils import run_bass_kernel_spmd

F32 = mybir.dt.float32
F32R = mybir.dt.float32r
U32 = mybir.dt.uint32
EXP = mybir.ActivationFunctionType.Exp
P = 128
ONE_F32_BITS = 0x3F800000


def build_nc(L=2048, DIN=1024, HG=8, HD=64, DOUT=1024, reps=1):
    """Build the per-core Bass program (SPMD across the 8 cores)."""
    DH = HG * HD               # head-group width (q, k, v each)
    KC = DIN // P              # contraction chunks for the projections
    NKC = L // P               # key chunks
    QHW = min(1024, L)         # q-block width for attention
    NQH = L // QHW
    NW = min(512, L)           # matmul moving width
    SCALE = 1.0 / np.sqrt(HD)
    MQK = 2 * DH // P          # qk^T row chunks
    VW = HG * (HD + 1)         # v tile width (per-head 64 cols + ones col)

    nc = bacc.Bacc("TRN2", target_bir_lowering=False, debug=False)

    xt = nc.dram_tensor("xt", [DIN, L], F32R, kind="ExternalInput")
    wqk = nc.dram_tensor("wqk", [DIN, 2 * DH], F32R, kind="ExternalInput")
    wv = nc.dram_tensor("wv", [DIN, DH], F32R, kind="ExternalInput")
    bqk = nc.dram_tensor("bqk", [2 * DH], F32, kind="ExternalInput")
    bv = nc.dram_tensor("bv", [1, DH], F32R, kind="ExternalInput")
    wo = nc.dram_tensor("wo", [DH, DOUT], F32R, kind="ExternalInput")
    out_t = nc.dram_tensor("out_t", [DOUT, L], F32, kind="ExternalOutput")
    # tiny pass-through so the benchmark harness has a cheap in/out pair
    tok = nc.dram_tensor("tok", [1, 4], F32, kind="ExternalInput")
    tok_out = nc.dram_tensor("tok_out", [1, 4], F32, kind="ExternalOutput")

    def emit(tc, x):
        """Emit one full pipeline; x is a name prefix."""
        with tc.tile_pool(name=x + "persist", bufs=1) as persist:
            qkt_sb = [persist.tile([P, L], F32R, tag=f"qkt{m}",
                                   name=f"{x}qkt{m}") for m in range(MQK)]
            vt_sb = [persist.tile([P, VW], F32R, tag=f"vt{k}",
                                  name=f"{x}vt{k}") for k in range(NKC)]
            bqk_sb = persist.tile([P, MQK], F32, tag="bqk", name=x + "bqk_sb")
            bv_sb = persist.tile([1, DH], F32R, tag="bv", name=x + "bv_sb")
            ones_sb = persist.tile([P, P], F32R, tag="ones", name=x + "ones_sb")

            nc.sync.dma_start(bqk_sb[:], bqk.rearrange("(m p) -> p m", p=P))
            nc.sync.dma_start(bv_sb[:], bv[:])
            nc.vector.memset(ones_sb[:].bitcast(U32), ONE_F32_BITS)
            for k in range(NKC):
                for h in range(HG):
                    c0 = h * (HD + 1) + HD
                    nc.vector.memset(vt_sb[k][:, c0:c0 + 1].bitcast(U32),
                                     ONE_F32_BITS)

            # ---------------- phase 1: projections ----------------
            with (
                tc.tile_pool(name=x + "wqk_pool", bufs=1) as wqk_pool,
                tc.tile_pool(name=x + "wv_pool", bufs=1) as wv_pool,
                tc.tile_pool(name=x + "xt_pool", bufs=2 * KC) as xt_pool,
                tc.tile_pool(name=x + "ps1", bufs=4, space="PSUM") as ps1,
            ):
                wqk_sb = [wqk_pool.tile([P, 2 * DH], F32R, tag=f"wqk{k}",
                                        name=f"{x}wqk{k}") for k in range(KC)]
                wv_sb = [wv_pool.tile([P, DH], F32R, tag=f"wv{k}",
                                      name=f"{x}wv{k}") for k in range(KC)]
                for k in range(KC):
                    nc.sync.dma_start(wqk_sb[k][:], wqk[k * P:(k + 1) * P, :])
                    nc.sync.dma_start(wv_sb[k][:], wv[k * P:(k + 1) * P, :])

                for xq in range(L // NW):
                    xs = slice(xq * NW, (xq + 1) * NW)
                    xts = []
                    for k in range(KC):
                        t = xt_pool.tile([P, NW], F32R, tag="xt",
                                         name=f"{x}xt{xq}_{k}")
                        nc.sync.dma_start(t[:], xt[k * P:(k + 1) * P, xs])
                        xts.append(t)
                    # (q|k)^T chunks: lhsT = wqk, rhs = x^T
                    for m in range(MQK):
                        ps = ps1.tile([P, NW], F32, tag="ps1",
                                      name=f"{x}pqk{xq}_{m}")
                        for k in range(KC):
                            nc.tensor.matmul(
                                ps[:], wqk_sb[k][:, m * P:(m + 1) * P],
                                xts[k][:], start=(k == 0), stop=(k == KC - 1))
                        nc.vector.tensor_scalar_add(
                            qkt_sb[m][:, xs], ps[:], bqk_sb[:, m:m + 1])
                    # v natural: lhsT = x^T chunk, rhs = wv (+ ones x bv)
                    for rc in range(NW // P):
                        kr = (xq * NW) // P + rc
                        ps = ps1.tile([P, DH], F32, tag="ps1", name=f"{x}pv{kr}")
                        for k in range(KC):
                            nc.tensor.matmul(
                                ps[:], xts[k][:, rc * P:(rc + 1) * P],
                                wv_sb[k][:], start=(k == 0), stop=False)
                        nc.tensor.matmul(
                            ps[:], ones_sb[:1, :P], bv_sb[:],
                            start=False, stop=True)
                        for h in range(HG):
                            nc.vector.tensor_copy(
                                vt_sb[kr][:, h * (HD + 1):h * (HD + 1) + HD],
                                ps[:, h * HD:(h + 1) * HD])

            # ---------------- phase 2: attention ----------------
            with tc.tile_pool(name=x + "ao_pool", bufs=1) as ao_pool:
                ao_sb = [ao_pool.tile([P, L], F32R, tag=f"ao{j}",
                                      name=f"{x}ao{j}")
                         for j in range(DH // P)]
                with (
                    tc.tile_pool(name=x + "pt_pool", bufs=3) as pt_pool,
                    tc.tile_pool(name=x + "zt_pool", bufs=2) as zt_pool,
                    tc.tile_pool(name=x + "rb_pool", bufs=2) as rb_pool,
                    tc.tile_pool(name=x + "at_pool", bufs=2) as at_pool,
                    tc.tile_pool(name=x + "ps2", bufs=2, space="PSUM") as ps2,
                    tc.tile_pool(name=x + "psav", bufs=2, space="PSUM") as psav,
                ):
                    for h in range(HG):
                        j, po = h // 2, HD * (h % 2)
                        hv = slice(h * (HD + 1), (h + 1) * (HD + 1))
                        for qh in range(NQH):
                            q0 = qh * QHW
                            av = psav.tile([HD + 1, QHW], F32, tag="av",
                                           name=f"{x}av{h}_{qh}")
                            for kc in range(NKC):
                                sp = ps2.tile([P, QHW], F32, tag="sc",
                                              name=f"{x}sp{h}_{qh}_{kc}")
                                for qs in range(QHW // NW):
                                    mq = slice(qs * NW, (qs + 1) * NW)
                                    nc.tensor.matmul(
                                        sp[:, mq],
                                        qkt_sb[DH // P + j][po:po + HD,
                                                            kc * P:(kc + 1) * P],
                                        qkt_sb[j][po:po + HD,
                                                  q0 + qs * NW:q0 + (qs + 1) * NW],
                                        start=True, stop=True)
                                pt = pt_pool.tile([P, QHW], F32R, tag="pt",
                                                  name=f"{x}pt{h}_{qh}_{kc}")
                                nc.scalar.activation(pt[:], sp[:], EXP,
                                                     scale=float(SCALE))
                                for qs in range(QHW // NW):
                                    mq = slice(qs * NW, (qs + 1) * NW)
                                    nc.tensor.matmul(
                                        av[:, mq], vt_sb[kc][:, hv], pt[:, mq],
                                        start=(kc == 0), stop=(kc == NKC - 1))
                            # normalize by Z (= row HD of av)
                            zt = zt_pool.tile([P, QHW], F32R, tag="zt",
                                              name=f"{x}zt{h}_{qh}")
                            nc.vector.tensor_copy(zt[HD:HD + 1, :],
                                                  av[HD:HD + 1, :])
                            bc = ps2.tile([HD, QHW], F32, tag="sc",
                                          name=f"{x}bc{h}_{qh}")
                            for qs in range(QHW // NW):
                                mq = slice(qs * NW, (qs + 1) * NW)
                                nc.tensor.matmul(
                                    bc[:, mq], ones_sb[HD:HD + 1, :HD],
                                    zt[HD:HD + 1, mq], start=True, stop=True)
                            rb = rb_pool.tile([HD, QHW], F32, tag="rb",
                                              name=f"{x}rb{h}_{qh}")
                            nc.vector.reciprocal(rb[:], bc[:])
                            if po == 0:
                                nc.vector.tensor_mul(
                                    ao_sb[j][0:HD, q0:q0 + QHW],
                                    av[0:HD, :], rb[:])
                            else:
                                at = at_pool.tile([HD, QHW], F32R, tag="at",
                                                  name=f"{x}at{h}_{qh}")
                                nc.vector.tensor_mul(at[:], av[0:HD, :], rb[:])
                                nc.sync.dma_start(
                                    ao_sb[j][po:po + HD, q0:q0 + QHW], at[:])

                # ---------------- phase 3: output projection ----------------
                with (
                    tc.tile_pool(name=x + "wo_pool", bufs=1) as wo_pool,
                    tc.tile_pool(name=x + "ot_pool", bufs=4) as ot_pool,
                    tc.tile_pool(name=x + "ps3", bufs=4, space="PSUM") as ps3,
                ):
                    wo_sb = [wo_pool.tile([P, DOUT], F32R, tag=f"wo{d}",
                                          name=f"{x}wo{d}")
                             for d in range(DH // P)]
                    for d in range(DH // P):
                        nc.sync.dma_start(wo_sb[d][:], wo[d * P:(d + 1) * P, :])
                    for oc in range(DOUT // P):
                        for qb in range(L // NW):
                            qs_ = slice(qb * NW, (qb + 1) * NW)
                            ps = ps3.tile([P, NW], F32, tag="ps3",
                                          name=f"{x}po{oc}_{qb}")
                            for d in range(DH // P):
                                nc.tensor.matmul(
                                    ps[:], wo_sb[d][:, oc * P:(oc + 1) * P],
                                    ao_sb[d][:, qs_],
                                    start=(d == 0), stop=(d == DH // P - 1))
                            ot = ot_pool.tile([P, NW], F32, tag="ot",
                                              name=f"{x}ot{oc}_{qb}")
                            nc.vector.tensor_copy(ot[:], ps[:])
                            nc.sync.dma_start(out_t[oc * P:(oc + 1) * P, qs_],
                                              ot[:])

    with tile.TileContext(nc) as tc:
        with tc.tile_pool(name="tokp", bufs=1) as tokp:
            tok_sb = tokp.tile([1, 4], F32, tag="tok", name="tok_sb")
            nc.sync.dma_start(tok_sb[:], tok[:])
            nc.sync.dma_start(tok_out[:], tok_sb[:])
        for rep in range(reps):
            emit(tc, f"r{rep}_" if reps > 1 else "")

    nc.compile()
    return nc


_NC_CACHE = {}


def get_nc(**kw):
    key = tuple(sorted(kw.items()))
    if key not in _NC_CACHE:
        _NC_CACHE[key] = build_nc(**kw)
    return _NC_CACHE[key]


def make_in_maps(data, W_qkv, b_qkv, W_out, n_cores=8):
    """Shard full inputs: core c -> (batch c//2, head-group c%2)."""
    data = np.asarray(data, np.float32)
    W_qkv = np.asarray(W_qkv, np.float32)
    b_qkv = np.asarray(b_qkv, np.float32)
    W_out = np.asarray(W_out, np.float32)
    N = data.shape[0]
    DIM = W_out.shape[0]
    G = n_cores // N                      # head groups
    DH = DIM // G
    in_maps = []
    for c in range(n_cores):
        n, g = c // G, c % G
        sq = slice(g * DH, (g + 1) * DH)
        sk = slice(DIM + g * DH, DIM + (g + 1) * DH)
        sv = slice(2 * DIM + g * DH, 2 * DIM + (g + 1) * DH)
        in_maps.append({
            "xt": np.ascontiguousarray(data[n].T),
            "wqk": np.ascontiguousarray(
                np.concatenate([W_qkv[:, sq], W_qkv[:, sk]], axis=1)),
            "wv": np.ascontiguousarray(W_qkv[:, sv]),
            "bqk": np.ascontiguousarray(
                np.concatenate([b_qkv[sq], b_qkv[sk]])),
            "bv": np.ascontiguousarray(b_qkv[sv][None, :]),
            "wo": np.ascontiguousarray(W_out[g * DH:(g + 1) * DH, :]),
            "tok": np.zeros((1, 4), np.float32),
        })
    return in_maps


def kernel(data, W_qkv, b_qkv, W_out, b_out):
    data = np.asarray(data, np.float32)
    b_out = np.asarray(b_out, np.float32)
    N, L, DIN = data.shape
    DIM = np.asarray(W_out).shape[0]
    G = 8 // N
    HD = 64
    nc = get_nc(L=L, DIN=DIN, HG=DIM // HD // G, HD=HD, DOUT=DIM)
    in_maps = make_in_maps(data, W_qkv, b_qkv, W_out)
    res = run_bass_kernel_spmd(nc, in_maps, core_ids=list(range(8)))
    out = np.empty((N, L, DIM), np.float32)
    for n in range(N):
        acc = res.results[G * n]["out_t"].copy()
        for g in range(1, G):
            acc += res.results[G * n + g]["out_t"]
        out[n] = acc.T + b_out
    return out
